# revision 1
# baseline (speedup 1.0000x reference)
"""Trainium2 Bass kernel for nn_MixtureOfAgents.

Contract: kernel(**inputs) takes FULL unsharded inputs (numpy) and returns the
FULL output [4, 4096, 768] float32.

Strategy:
  - The reference has a quirk: for each of the TOP_K=2 steps, ONE scalar agent
    id (top_i[0, -1, k] -- the top-k'th agent of the last token of batch 0)
    selects the expert weight block used for ALL tokens.  The host computes
    that scalar (a 10x768 dot product) and slices/transposes the selected
    expert blocks; the device does everything else.
  - Data-parallel over tokens: 8 cores x 2048 tokens.  Weights replicated.
  - On device, activations live in transposed layout [C, tok] (channels on
    partitions); x is transposed on-chip via PE transposes.  Big matmuls run
    as float32r (full-rate fp32 on the PE).  Per-token routing (softmax over
    10 agents, top-2 + renorm, expert-pair softmax, role-emb one-hot gather)
    is computed on-chip in fp32.
"""

import numpy as np

# ---- problem constants (hardcoded; kernel.py must be self-contained) ----
N_CORES = 8
B, T, C = 4, 4096, 768
TOK = B * T              # 16384
TPC = TOK // N_CORES     # 2048 tokens per core
CT = C // 128            # 6 c-tiles
FFN = 2048
FT = FFN // 128          # 16 f-tiles per expert
A = 10                   # n_agents
EPA = 2                  # experts per agent
NG = A + 2 * EPA         # 14 packed gate columns (10 agent + 2x2 expert)
TOPK = 2
CHUNK = 1024             # tokens per on-chip chunk
NCHUNK = TPC // CHUNK    # 2
NBLK = CHUNK // 128      # 8 token-blocks per chunk

_CACHE = {}


def _build_module(mm_dtype="f32r"):
    import concourse.bass as bass
    import concourse.bacc as bacc
    import concourse.mybir as mybir
    import concourse.tile as tile
    from concourse.masks import make_identity
    from contextlib import ExitStack

    f32 = mybir.dt.float32
    f32r = mybir.dt.float32r
    AF = mybir.ActivationFunctionType
    OP = mybir.AluOpType

    bf16 = mybir.dt.bfloat16
    fmm = {"f32r": f32r, "f32": f32, "bf16": bf16}[mm_dtype]

    def mmv(ap):
        return ap  # tiles already carry the matmul dtype

    nc = bacc.Bacc(target_bir_lowering=False)
    xs = nc.dram_tensor("xs", [TPC, C], f32, kind="ExternalInput")
    gt = nc.dram_tensor("gt", [C, NG], fmm, kind="ExternalInput")
    role = nc.dram_tensor("role", [A, C], fmm, kind="ExternalInput")
    w1t = nc.dram_tensor("w1t", [TOPK, C, EPA * FFN], fmm, kind="ExternalInput")
    w3t = nc.dram_tensor("w3t", [TOPK, C, EPA * FFN], fmm, kind="ExternalInput")
    w2t = nc.dram_tensor("w2t", [TOPK, EPA * FFN, C], fmm, kind="ExternalInput")
    out = nc.dram_tensor("out", [TPC, C], f32, kind="ExternalOutput")

    with ExitStack() as ctx:
        tc = ctx.enter_context(tile.TileContext(nc))
        const = ctx.enter_context(tc.tile_pool(name="const", bufs=1))
        persist = ctx.enter_context(tc.tile_pool(name="persist", bufs=1))
        stage = ctx.enter_context(tc.tile_pool(name="stage", bufs=2))
        w13p = ctx.enter_context(tc.tile_pool(name="w13p", bufs=8))
        w2p = ctx.enter_context(tc.tile_pool(name="w2p", bufs=8))
        rpool = ctx.enter_context(tc.tile_pool(name="rpool", bufs=2))
        tmpp = ctx.enter_context(tc.tile_pool(name="tmpp", bufs=1))
        psA = ctx.enter_context(tc.tile_pool(name="psA", bufs=6, space="PSUM"))
        psB = ctx.enter_context(tc.tile_pool(name="psB", bufs=2, space="PSUM"))

        ident = const.tile([128, 128], f32)
        make_identity(nc, ident)
        gt_sb = const.tile([128, CT, NG], fmm)
        nc.sync.dma_start(out=gt_sb, in_=gt[:, :].rearrange("(g p) n -> p g n", p=128))
        role_sb = const.tile([A, C], fmm)
        nc.sync.dma_start(out=role_sb, in_=role[:, :])

        def phase_xt(ch):
            tok0 = ch * CHUNK
            xT = persist.tile([128, CT, CHUNK], fmm, tag="xT", name=f"xT_{ch}")
            for blk in range(NBLK):
                xb = stage.tile([128, C], f32, tag="xblk", name=f"xb_{ch}_{blk}")
                nc.sync.dma_start(out=xb, in_=xs[tok0 + blk * 128: tok0 + (blk + 1) * 128, :])
                for c in range(CT):
                    pt = psB.tile([128, 128], f32, tag="ps_small", name=f"ptx_{ch}_{blk}_{c}")
                    nc.tensor.transpose(pt, xb[:, c * 128:(c + 1) * 128], ident)
                    nc.vector.tensor_copy(xT[:, c, blk * 128:(blk + 1) * 128], pt)
            return xT

        def phase_routing(ch, xT):
            # logits.T [NG, CHUNK] = gt.T @ x.T
            Lsb = persist.tile([NG, CHUNK], f32, tag="L", name=f"L_{ch}")
            for half in range(2):
                pl = psB.tile([NG, 512], f32, tag="ps_small", name=f"pl_{ch}_{half}")
                for c in range(CT):
                    nc.tensor.matmul(
                        pl, gt_sb[:, c, :], xT[:, c, half * 512:(half + 1) * 512],
                        start=(c == 0), stop=(c == CT - 1),
                    )
                nc.vector.tensor_copy(Lsb[:, half * 512:(half + 1) * 512], pl)

            RT_oh = [persist.tile([A, CHUNK], fmm, tag=f"rtoh{k}", name=f"rtoh{k}_{ch}") for k in range(TOPK)]
            RT_g = [persist.tile([1, CHUNK], f32, tag=f"rtg{j}", name=f"rtg{j}_{ch}") for j in range(4)]

            for blk in range(NBLK):
                bsl = slice(blk * 128, (blk + 1) * 128)
                pt = psB.tile([128, NG], f32, tag="ps_small", name=f"ptl_{ch}_{blk}")
                nc.tensor.transpose(pt, Lsb[:, bsl], ident[:NG, :NG])
                lt = rpool.tile([128, NG], f32, tag="lt", name=f"lt_{ch}_{blk}")
                nc.vector.tensor_copy(lt, pt)

                rt = rpool.tile([128, 2 * A + 4], f32, tag="rt", name=f"rt_{ch}_{blk}")
                aw = rpool.tile([128, A], f32, tag="aw", name=f"aw_{ch}_{blk}")
                awm = rpool.tile([128, A], f32, tag="awm", name=f"awm_{ch}_{blk}")
                t10 = rpool.tile([128, A], f32, tag="t10", name=f"t10_{ch}_{blk}")
                mx = rpool.tile([128, 1], f32, tag="mx", name=f"mx_{ch}_{blk}")
                nmx = rpool.tile([128, 1], f32, tag="nmx", name=f"nmx_{ch}_{blk}")
                ssum = rpool.tile([128, 1], f32, tag="ssum", name=f"ssum_{ch}_{blk}")
                rs = rpool.tile([128, 1], f32, tag="rs", name=f"rs_{ch}_{blk}")
                m1 = rpool.tile([128, 1], f32, tag="m1", name=f"m1_{ch}_{blk}")
                m2 = rpool.tile([128, 1], f32, tag="m2", name=f"m2_{ch}_{blk}")
                dn = rpool.tile([128, 1], f32, tag="dn", name=f"dn_{ch}_{blk}")
                tw = [rpool.tile([128, 1], f32, tag=f"tw{k}", name=f"tw{k}_{ch}_{blk}") for k in range(TOPK)]

                # agent softmax over cols 0:A
                nc.vector.reduce_max(mx, lt[:, 0:A], axis=mybir.AxisListType.X)
                nc.vector.tensor_scalar_mul(nmx, mx, -1.0)
                nc.scalar.activation(aw, lt[:, 0:A], AF.Exp, bias=nmx, scale=1.0)
                nc.vector.reduce_sum(ssum, aw, axis=mybir.AxisListType.X)
                nc.vector.reciprocal(rs, ssum)
                nc.vector.tensor_scalar_mul(aw, aw, rs)
                # top-1 / top-2
                nc.vector.reduce_max(m1, aw, axis=mybir.AxisListType.X)
                nc.vector.tensor_scalar(rt[:, 0:A], aw, m1, None, op0=OP.is_equal)
                nc.vector.tensor_tensor(t10, aw, rt[:, 0:A], op=OP.mult)
                nc.vector.tensor_tensor(awm, aw, t10, op=OP.subtract)
                nc.vector.reduce_max(m2, awm, axis=mybir.AxisListType.X)
                nc.vector.tensor_scalar(rt[:, A:2 * A], awm, m2, None, op0=OP.is_equal)
                # renormalized top weights
                nc.vector.tensor_tensor(dn, m1, m2, op=OP.add)
                nc.vector.tensor_scalar_add(dn, dn, 1e-6)
                nc.vector.reciprocal(dn, dn)
                nc.vector.tensor_tensor(tw[0], m1, dn, op=OP.mult)
                nc.vector.tensor_tensor(tw[1], m2, dn, op=OP.mult)
                # expert-pair softmax per k; g = tw_k * ae
                for k in range(TOPK):
                    ca = lt[:, A + 2 * k: A + 2 * k + 1]
                    cb = lt[:, A + 2 * k + 1: A + 2 * k + 2]
                    mk = rpool.tile([128, 1], f32, tag="mk", name=f"mk_{ch}_{blk}_{k}")
                    nmk = rpool.tile([128, 1], f32, tag="nmk", name=f"nmk_{ch}_{blk}_{k}")
                    ea = rpool.tile([128, 1], f32, tag="ea", name=f"ea_{ch}_{blk}_{k}")
                    eb = rpool.tile([128, 1], f32, tag="eb", name=f"eb_{ch}_{blk}_{k}")
                    es = rpool.tile([128, 1], f32, tag="es", name=f"es_{ch}_{blk}_{k}")
                    nc.vector.tensor_tensor(mk, ca, cb, op=OP.max)
                    nc.vector.tensor_scalar_mul(nmk, mk, -1.0)
                    nc.scalar.activation(ea, ca, AF.Exp, bias=nmk, scale=1.0)
                    nc.scalar.activation(eb, cb, AF.Exp, bias=nmk, scale=1.0)
                    nc.vector.tensor_tensor(es, ea, eb, op=OP.add)
                    nc.vector.reciprocal(es, es)
                    nc.vector.tensor_tensor(ea, ea, es, op=OP.mult)
                    nc.vector.tensor_tensor(eb, eb, es, op=OP.mult)
                    nc.vector.tensor_tensor(rt[:, 2 * A + 2 * k: 2 * A + 2 * k + 1], tw[k], ea, op=OP.mult)
                    nc.vector.tensor_tensor(rt[:, 2 * A + 2 * k + 1: 2 * A + 2 * k + 2], tw[k], eb, op=OP.mult)

                # transpose back: one-hots per k, g columns
                for k in range(TOPK):
                    po = psB.tile([A, 128], f32, tag="ps_small", name=f"po_{ch}_{blk}_{k}")
                    nc.tensor.transpose(po, rt[:, k * A:(k + 1) * A], ident)
                    nc.vector.tensor_copy(RT_oh[k][:, bsl], po)
                for j in range(4):
                    pg = psB.tile([1, 128], f32, tag="ps_small", name=f"pg_{ch}_{blk}_{j}")
                    nc.tensor.transpose(pg, rt[:, 2 * A + j: 2 * A + j + 1], ident)
                    nc.vector.tensor_copy(RT_g[j][:, bsl], pg)

            # broadcast g rows across partitions: B_j [128, CHUNK]
            Bt = [persist.tile([128, CHUNK], f32, tag=f"B{j}", name=f"B{j}_{ch}") for j in range(4)]
            for j in range(4):
                nc.gpsimd.partition_broadcast(Bt[j], RT_g[j][:, :])
            return RT_oh, Bt

        def phase_main(ch, xT, RT_oh, Bt):
            acc = persist.tile([128, CT, CHUNK], f32, tag="acc", name=f"acc_{ch}")
            hid = persist.tile([128, FT, CHUNK], fmm, tag="hid", name=f"hid_{ch}")
            for k in range(TOPK):
                for e in range(EPA):
                    # mm1 + mm3 + silu*mul -> hidden [FFN, CHUNK]
                    for f in range(FT):
                        col = e * FFN + f * 128
                        w1f = w13p.tile([128, CT, 128], fmm, tag="w13", name=f"w1f_{ch}_{k}_{e}_{f}")
                        nc.sync.dma_start(
                            out=w1f, in_=w1t[k, :, col:col + 128].rearrange("(g p) f -> p g f", p=128))
                        w3f = w13p.tile([128, CT, 128], fmm, tag="w13", name=f"w3f_{ch}_{k}_{e}_{f}")
                        nc.sync.dma_start(
                            out=w3f, in_=w3t[k, :, col:col + 128].rearrange("(g p) f -> p g f", p=128))
                        ph1 = [psA.tile([128, 512], f32, tag="ps_main", name=f"ph1_{ch}_{k}_{e}_{f}_{tq}")
                               for tq in range(2)]
                        ph3 = [psA.tile([128, 512], f32, tag="ps_main", name=f"ph3_{ch}_{k}_{e}_{f}_{tq}")
                               for tq in range(2)]
                        for tq in range(2):
                            tsl = slice(tq * 512, (tq + 1) * 512)
                            for c in range(CT):
                                nc.tensor.matmul(
                                    ph1[tq], mmv(w1f[:, c, :]), mmv(xT[:, c, tsl]),
                                    start=(c == 0), stop=(c == CT - 1))
                        for tq in range(2):
                            tsl = slice(tq * 512, (tq + 1) * 512)
                            for c in range(CT):
                                nc.tensor.matmul(
                                    ph3[tq], mmv(w3f[:, c, :]), mmv(xT[:, c, tsl]),
                                    start=(c == 0), stop=(c == CT - 1))
                        for tq in range(2):
                            tsl = slice(tq * 512, (tq + 1) * 512)
                            nc.scalar.activation(hid[:, f, tsl], ph1[tq], AF.Silu)
                            nc.vector.tensor_tensor(hid[:, f, tsl], hid[:, f, tsl], ph3[tq], op=OP.mult)

                    # mm2: y[c] = sum_f w2.T @ hid ; combine into acc
                    for c in range(CT):
                        py = [psA.tile([128, 512], f32, tag="ps_main", name=f"py_{ch}_{k}_{e}_{c}_{tq}")
                              for tq in range(2)]
                        for f in range(FT):
                            row = e * FFN + f * 128
                            w2f = w2p.tile([128, 128], fmm, tag="w2", name=f"w2f_{ch}_{k}_{e}_{c}_{f}")
                            nc.sync.dma_start(
                                out=w2f, in_=w2t[k, row:row + 128, c * 128:(c + 1) * 128])
                            for tq in range(2):
                                tsl = slice(tq * 512, (tq + 1) * 512)
                                nc.tensor.matmul(
                                    py[tq], mmv(w2f), mmv(hid[:, f, tsl]),
                                    start=(f == 0), stop=(f == FT - 1))
                        # tmp = B_{k,e} * y ; tmp2 = (pe + 1) * tmp ; acc (+)= tmp2
                        for tq in range(2):
                            tsl = slice(tq * 512, (tq + 1) * 512)
                            t1 = tmpp.tile([128, 512], f32, tag="t1", name=f"t1_{ch}_{k}_{e}_{c}_{tq}")
                            nc.vector.tensor_tensor(t1, Bt[2 * k + e][:, tsl], py[tq], op=OP.mult)
                            pe_ = psB.tile([128, 512], f32, tag="ps_small", name=f"pe_{ch}_{k}_{e}_{c}_{tq}")
                            nc.tensor.matmul(
                                pe_, mmv(role_sb[:, c * 128:(c + 1) * 128]),
                                mmv(RT_oh[k][:, tsl]), start=True, stop=True)
                            if k == 0 and e == 0:
                                nc.vector.scalar_tensor_tensor(
                                    out=acc[:, c, tsl], in0=pe_, scalar=1.0, in1=t1,
                                    op0=OP.add, op1=OP.mult)
                            else:
                                t2 = tmpp.tile([128, 512], f32, tag="t2", name=f"t2_{ch}_{k}_{e}_{c}_{tq}")
                                nc.vector.scalar_tensor_tensor(
                                    out=t2, in0=pe_, scalar=1.0, in1=t1,
                                    op0=OP.add, op1=OP.mult)
                                nc.vector.tensor_tensor(acc[:, c, tsl], acc[:, c, tsl], t2, op=OP.add)
            return acc

        def phase_out(ch, acc):
            tok0 = ch * CHUNK
            for blk in range(NBLK):
                bsl = slice(blk * 128, (blk + 1) * 128)
                ob = stage.tile([128, C], f32, tag="outblk", name=f"ob_{ch}_{blk}")
                for c in range(CT):
                    pt = psB.tile([128, 128], f32, tag="ps_small", name=f"pto_{ch}_{blk}_{c}")
                    nc.tensor.transpose(pt, acc[:, c, bsl], ident)
                    nc.vector.tensor_copy(ob[:, c * 128:(c + 1) * 128], pt)
                nc.sync.dma_start(out=out[tok0 + blk * 128: tok0 + (blk + 1) * 128, :], in_=ob)

        # pipeline: the output phase of chunk ch is emitted after chunk ch+1's
        # x-transpose + routing phases, so those fill the PE while the DVE
        # drains chunk ch's combines.
        prev = None
        for ch in range(NCHUNK):
            xT = phase_xt(ch)
            RT_oh, Bt = phase_routing(ch, xT)
            if prev is not None:
                phase_out(*prev)
            prev = (ch, phase_main(ch, xT, RT_oh, Bt))
        phase_out(*prev)

    nc.compile()
    return nc


def _get_nc(mm_dtype="f32r"):
    key = ("nc", mm_dtype)
    if key not in _CACHE:
        _CACHE[key] = _build_module(mm_dtype)
    return _CACHE[key]


def _enable_jax_compile_cache():
    try:
        import jax
        jax.config.update("jax_compilation_cache_dir", "/tmp/jax_kernel_cache")
        jax.config.update("jax_persistent_cache_min_compile_time_secs", 1.0)
    except Exception:
        pass


def kernel(x, agent_gate_w, expert_gate_w, role_emb, w1, w2, w3,
           _trace=False, _dtype="f32r"):
    from concourse.bass_utils import run_bass_kernel_spmd

    _enable_jax_compile_cache()

    x = np.asarray(x, dtype=np.float32)
    agent_gate_w = np.asarray(agent_gate_w, dtype=np.float32)
    expert_gate_w = np.asarray(expert_gate_w, dtype=np.float32)
    role_emb = np.asarray(role_emb, dtype=np.float32)
    w1 = np.asarray(w1, dtype=np.float32)
    w2 = np.asarray(w2, dtype=np.float32)
    w3 = np.asarray(w3, dtype=np.float32)

    xf = np.ascontiguousarray(x.reshape(TOK, C))

    # host scalar routing: the reference's agent_id = top_i[0, -1, k]
    logits = xf[T - 1] @ agent_gate_w.T          # token [0, -1] -> flat index T-1
    order = np.argsort(-logits, kind="stable")
    sel = [int(order[0]) * EPA, int(order[1]) * EPA]

    gt = np.ascontiguousarray(
        np.concatenate([agent_gate_w,
                        expert_gate_w[sel[0]:sel[0] + EPA],
                        expert_gate_w[sel[1]:sel[1] + EPA]], axis=0).T)     # [C, NG]
    role_s = np.ascontiguousarray(0.1 * role_emb)                           # [A, C]
    w1tp = np.stack([w1[s:s + EPA].reshape(EPA * FFN, C).T for s in sel])   # [2, C, 2F]
    w3tp = np.stack([w3[s:s + EPA].reshape(EPA * FFN, C).T for s in sel])
    w2tp = np.stack([w2[s:s + EPA].transpose(0, 2, 1).reshape(EPA * FFN, C) for s in sel])
    w1tp = np.ascontiguousarray(w1tp)
    w3tp = np.ascontiguousarray(w3tp)
    w2tp = np.ascontiguousarray(w2tp)

    if _dtype == "bf16":
        import ml_dtypes
        cast = lambda a: np.ascontiguousarray(a.astype(ml_dtypes.bfloat16))
        gt, role_s = cast(gt), cast(role_s)
        w1tp, w3tp, w2tp = cast(w1tp), cast(w3tp), cast(w2tp)
    nc = _get_nc(_dtype)
    in_maps = []
    for i in range(N_CORES):
        in_maps.append({
            "xs": np.ascontiguousarray(xf[i * TPC:(i + 1) * TPC]),
            "gt": gt, "role": role_s,
            "w1t": w1tp, "w3t": w3tp, "w2t": w2tp,
        })
    res = run_bass_kernel_spmd(nc, in_maps, core_ids=list(range(N_CORES)),
                               trace=_trace)
    _CACHE["last_results"] = res
    out = np.concatenate([r["out"] for r in res.results], axis=0)
    return out.reshape(B, T, C)



# revision 35
# speedup vs baseline: 1.2971x; 1.2971x over previous
"""Trainium2 Bass kernel for nn_MixtureOfAgents.

Contract: kernel(**inputs) takes FULL unsharded inputs (numpy) and returns the
FULL output [4, 4096, 768] float32.

Strategy (v2):
  - Reference quirk: for each of TOP_K=2 steps, ONE scalar agent id
    (top_i[0, -1, k]) selects the expert pair used for ALL tokens.  The host
    computes the full per-token routing (agent softmax, top-2 renorm, expert
    pair softmax -> 4 gate rows g, plus agent one-hots) and slices the 4
    selected expert FFN blocks.  The device runs only the dense pipeline:
    transpose x, 4x (mm1/mm3 -> silu*mul -> mm2 -> combine), transpose out.
  - Data-parallel over tokens: 8 cores x 2048 tokens, weights replicated.
  - All matmul operands in bf16 (weights/x/hidden); PSUM accumulation fp32;
    combine arithmetic fp32.  Weights are pre-laid on host so each SBUF tile
    is one contiguous DMA.
  - mm1/mm3 stream two 512-token halves per stationary weight load.
"""

import numpy as np

# ---- problem constants (hardcoded; kernel.py must be self-contained) ----
N_CORES = 8
B, T, C = 4, 4096, 768
TOK = B * T              # 16384
TPC = TOK // N_CORES     # 2048 tokens per core
CT = C // 128            # 6 c-tiles
FFN = 2048
FT = FFN // 128          # 16 f-tiles per expert
A = 10                   # n_agents
EPA = 2                  # experts per agent
TOPK = 2
NBLK = TPC // 128        # 16 token-blocks
NTQ = TPC // 512         # 4 token-quarters

_CACHE = {}


def _build_module():
    import concourse.bass as bass
    import concourse.bacc as bacc
    import concourse.mybir as mybir
    import concourse.tile as tile
    from concourse.masks import make_identity
    from contextlib import ExitStack

    f32 = mybir.dt.float32
    bf16 = mybir.dt.bfloat16
    AF = mybir.ActivationFunctionType
    OP = mybir.AluOpType

    nc = bacc.Bacc(target_bir_lowering=False)
    xs = nc.dram_tensor("xs", [128, CT, TPC], bf16, kind="ExternalInput")
    w1t = nc.dram_tensor("w1t", [TOPK, EPA, FT, 128, CT, 128], bf16,
                         kind="ExternalInput")
    w3t = nc.dram_tensor("w3t", [TOPK, EPA, FT, 128, CT, 128], bf16,
                         kind="ExternalInput")
    w2t = nc.dram_tensor("w2t", [TOPK, EPA, CT, 128, FT, 128], bf16,
                         kind="ExternalInput")
    role = nc.dram_tensor("role", [A, C], bf16, kind="ExternalInput")
    oh = nc.dram_tensor("oh", [TOPK, A, TPC], bf16, kind="ExternalInput")
    g4 = nc.dram_tensor("g4", [TOPK * EPA, 128, TPC], bf16, kind="ExternalInput")
    out = nc.dram_tensor("out", [CT, 128, TPC], f32, kind="ExternalOutput")
    import os as _os
    _dbg = _os.environ.get("KERNEL_DEBUG_DUMPS") == "1"
    if _dbg:
        dbg_hid = nc.dram_tensor("dbg_hid", [128, FT, TPC], bf16,
                                 kind="ExternalOutput")
        dbg_acc = nc.dram_tensor("dbg_acc", [128, CT, TPC], f32,
                                 kind="ExternalOutput")

    with ExitStack() as ctx:
        tc = ctx.enter_context(tile.TileContext(nc))
        const = ctx.enter_context(tc.tile_pool(name="const", bufs=1))
        persist = ctx.enter_context(tc.tile_pool(name="persist", bufs=1))
        w13p = ctx.enter_context(tc.tile_pool(name="w13p", bufs=6))
        w2p = ctx.enter_context(tc.tile_pool(name="w2p", bufs=2))
        btp = ctx.enter_context(tc.tile_pool(name="btp", bufs=2))
        tmpp = ctx.enter_context(tc.tile_pool(name="tmpp", bufs=2))
        psH = ctx.enter_context(tc.tile_pool(name="psH", bufs=4, space="PSUM"))
        psY = ctx.enter_context(tc.tile_pool(name="psY", bufs=3, space="PSUM"))
        psR = ctx.enter_context(tc.tile_pool(name="psR", bufs=1, space="PSUM"))

        role_sb = const.tile([A, C], bf16)
        nc.sync.dma_start(out=role_sb, in_=role[:, :])
        oh_sb = [const.tile([A, TPC], bf16, name=f"oh{k}") for k in range(TOPK)]
        for k in range(TOPK):
            nc.sync.dma_start(out=oh_sb[k], in_=oh[k, :, :])


        xT = persist.tile([128, CT, TPC], bf16, tag="xT", name="xT")
        hid = persist.tile([128, FT, TPC], bf16, tag="hid", name="hid")
        acc = persist.tile([128, CT, TPC], f32, tag="acc", name="acc")

        # ---- prologue: x arrives pre-transposed [C-part, c-tile, tok] ----
        nc.sync.dma_start(out=xT, in_=xs[:, :, :])

        # ---- main: 4 expert passes ----
        for k in range(TOPK):
            for e in range(EPA):
                bt = btp.tile([128, TPC], bf16, tag="bt", name=f"bt_{k}_{e}")
                nc.sync.dma_start(out=bt, in_=g4[2 * k + e, :, :])
                # mm1 + mm3 + silu*mul -> hid [FFN, TPC]
                for f in range(FT):
                    w1f = w13p.tile([128, CT, 128], bf16, tag="w13",
                                    name=f"w1f_{k}_{e}_{f}")
                    nc.sync.dma_start(out=w1f, in_=w1t[k, e, f, :, :, :])
                    w3f = w13p.tile([128, CT, 128], bf16, tag="w13",
                                    name=f"w3f_{k}_{e}_{f}")
                    nc.sync.dma_start(out=w3f, in_=w3t[k, e, f, :, :, :])
                    for half in range(2):
                        t0 = slice(half * 1024, half * 1024 + 512)
                        t1 = slice(half * 1024 + 512, half * 1024 + 1024)
                        ph1 = [psH.tile([128, 512], f32, tag="ps_h",
                                        name=f"ph1_{k}_{e}_{f}_{half}_{q}")
                               for q in range(2)]
                        ph3 = [psH.tile([128, 512], f32, tag="ps_h",
                                        name=f"ph3_{k}_{e}_{f}_{half}_{q}")
                               for q in range(2)]
                        # pair token-halves per stationary weight tile
                        for c in range(CT):
                            nc.tensor.matmul(ph1[0], w1f[:, c, :], xT[:, c, t0],
                                             start=(c == 0), stop=(c == CT - 1))
                            nc.tensor.matmul(ph1[1], w1f[:, c, :], xT[:, c, t1],
                                             start=(c == 0), stop=(c == CT - 1))
                        for c in range(CT):
                            nc.tensor.matmul(ph3[0], w3f[:, c, :], xT[:, c, t0],
                                             start=(c == 0), stop=(c == CT - 1))
                            nc.tensor.matmul(ph3[1], w3f[:, c, :], xT[:, c, t1],
                                             start=(c == 0), stop=(c == CT - 1))
                        for q, ts in ((0, t0), (1, t1)):
                            nc.scalar.activation(hid[:, f, ts], ph1[q], AF.Silu)
                            nc.vector.tensor_tensor(hid[:, f, ts], hid[:, f, ts],
                                                    ph3[q], op=OP.mult)

                if _dbg and k == 0 and e == 0:
                    nc.sync.dma_start(out=dbg_hid[:, :, :], in_=hid[:, :, :])
                # mm2 + combine into acc
                for c in range(CT):
                    w2f = w2p.tile([128, FT, 128], bf16, tag="w2",
                                   name=f"w2f_{k}_{e}_{c}")
                    nc.sync.dma_start(out=w2f, in_=w2t[k, e, c, :, :, :])
                    for tq in range(NTQ):
                        ts = slice(tq * 512, (tq + 1) * 512)
                        py = psY.tile([128, 512], f32, tag="ps_y",
                                      name=f"py_{k}_{e}_{c}_{tq}")
                        for f in range(FT):
                            nc.tensor.matmul(py, w2f[:, f, :], hid[:, f, ts],
                                             start=(f == 0), stop=(f == FT - 1))
                        # role factor for this (k, c, tq)
                        pr = psR.tile([128, 512], f32, tag="ps_r",
                                      name=f"pr_{k}_{e}_{c}_{tq}")
                        nc.tensor.matmul(pr, role_sb[:, c * 128:(c + 1) * 128],
                                         oh_sb[k][:, ts], start=True, stop=True)
                        t1_ = tmpp.tile([128, 512], f32, tag="t1",
                                        name=f"t1_{k}_{e}_{c}_{tq}")
                        nc.vector.tensor_tensor(t1_, bt[:, ts], py,
                                                op=OP.mult)
                        if k == 0 and e == 0:
                            nc.vector.scalar_tensor_tensor(
                                out=acc[:, c, ts], in0=pr, scalar=1.0, in1=t1_,
                                op0=OP.add, op1=OP.mult)
                        else:
                            nc.vector.scalar_tensor_tensor(
                                out=t1_, in0=pr, scalar=1.0, in1=t1_,
                                op0=OP.add, op1=OP.mult)
                            nc.vector.tensor_tensor(acc[:, c, ts], acc[:, c, ts],
                                                    t1_, op=OP.add)

                if _dbg and k == 0 and e == 0:
                    nc.sync.dma_start(out=dbg_acc[:, :, :], in_=acc[:, :, :])

        # ---- epilogue: store acc in [c-tile, C-part, tok] layout ----
        for c in range(CT):
            nc.sync.dma_start(out=out[c, :, :], in_=acc[:, c, :])

    nc.compile()
    return nc


def _get_nc():
    if "nc" not in _CACHE:
        _CACHE["nc"] = _build_module()
    return _CACHE["nc"]


def _enable_jax_compile_cache():
    try:
        import jax
        jax.config.update("jax_compilation_cache_dir", "/tmp/jax_kernel_cache")
        jax.config.update("jax_persistent_cache_min_compile_time_secs", 1.0)
    except Exception:
        pass


def _host_routing(xf, agent_gate_w, expert_gate_w):
    """Per-token gates exactly as the reference computes them (fp32)."""
    al = xf @ agent_gate_w.T                                    # [TOK, A]
    al = al - al.max(axis=1, keepdims=True)
    aw = np.exp(al)
    aw /= aw.sum(axis=1, keepdims=True)
    order = np.argsort(-aw, axis=1, kind="stable")              # [TOK, A]
    i_k = order[:, :TOPK]                                       # [TOK, 2]
    w_k = np.take_along_axis(aw, i_k, axis=1)                   # [TOK, 2]
    tw = w_k / (w_k.sum(axis=1, keepdims=True) + 1e-6)          # [TOK, 2]

    # scalar agent ids from token (b=0, t=T-1) -> flat row T-1
    sel = [int(i_k[T - 1, k]) * EPA for k in range(TOPK)]

    cols = [sel[0], sel[0] + 1, sel[1], sel[1] + 1]
    el = xf @ expert_gate_w[cols].T                             # [TOK, 4]
    g = np.empty((4, TOK), dtype=np.float32)
    for k in range(TOPK):
        pair = el[:, 2 * k:2 * k + 2]
        pair = pair - pair.max(axis=1, keepdims=True)
        ew = np.exp(pair)
        ew /= ew.sum(axis=1, keepdims=True)
        g[2 * k] = tw[:, k] * ew[:, 0]
        g[2 * k + 1] = tw[:, k] * ew[:, 1]

    onehot = np.zeros((TOPK, A, TOK), dtype=np.float32)
    for k in range(TOPK):
        onehot[k, i_k[:, k], np.arange(TOK)] = 1.0
    return sel, g, onehot


def kernel(x, agent_gate_w, expert_gate_w, role_emb, w1, w2, w3,
           _trace=False, _dtype="f32r"):
    import ml_dtypes
    from concourse.bass_utils import run_bass_kernel_spmd

    _enable_jax_compile_cache()
    bf16 = ml_dtypes.bfloat16

    x = np.asarray(x, dtype=np.float32)
    agent_gate_w = np.asarray(agent_gate_w, dtype=np.float32)
    expert_gate_w = np.asarray(expert_gate_w, dtype=np.float32)
    role_emb = np.asarray(role_emb, dtype=np.float32)
    w1 = np.asarray(w1, dtype=np.float32)
    w2 = np.asarray(w2, dtype=np.float32)
    w3 = np.asarray(w3, dtype=np.float32)

    xf = np.ascontiguousarray(x.reshape(TOK, C))
    sel, g, onehot = _host_routing(xf, agent_gate_w, expert_gate_w)

    rows = [sel[0], sel[0] + 1, sel[1], sel[1] + 1]
    # w1/w3 tiles: [ke, f, p, c, j] with value w[e, f*128+j, c*128+p]
    w1sel = w1[rows].reshape(4, FT, 128, CT, 128).transpose(0, 1, 4, 3, 2)
    w3sel = w3[rows].reshape(4, FT, 128, CT, 128).transpose(0, 1, 4, 3, 2)
    # w2 tiles: [ke, c, p, f, j] with value w2[e, c*128+j, f*128+p]
    w2sel = w2[rows].reshape(4, CT, 128, FT, 128).transpose(0, 1, 4, 3, 2)

    w1tp = np.ascontiguousarray(
        w1sel.reshape(TOPK, EPA, FT, 128, CT, 128).astype(bf16))
    w3tp = np.ascontiguousarray(
        w3sel.reshape(TOPK, EPA, FT, 128, CT, 128).astype(bf16))
    w2tp = np.ascontiguousarray(
        w2sel.reshape(TOPK, EPA, CT, 128, FT, 128).astype(bf16))
    role_s = np.ascontiguousarray((0.1 * role_emb).astype(bf16))
    oh_b = np.ascontiguousarray(onehot.astype(bf16))
    g_b = np.ascontiguousarray(g.astype(bf16))
    # x pre-transposed per core: [128 C-part, CT, TPC]
    xb = xf.astype(bf16).reshape(N_CORES, TPC, CT, 128)

    nc = _get_nc()
    in_maps = []
    for i in range(N_CORES):
        sl = slice(i * TPC, (i + 1) * TPC)
        in_maps.append({
            "xs": np.ascontiguousarray(xb[i].transpose(2, 1, 0)),
            "w1t": w1tp, "w3t": w3tp, "w2t": w2tp,
            "role": role_s,
            "oh": np.ascontiguousarray(oh_b[:, :, sl]),
            "g4": np.ascontiguousarray(
                np.broadcast_to(g_b[:, None, sl], (4, 128, TPC))),
        })
    res = run_bass_kernel_spmd(nc, in_maps, core_ids=list(range(N_CORES)),
                               trace=_trace)
    _CACHE["last_results"] = res
    # per-core out is [CT, 128, TPC]; reassemble to [TPC, C]
    out = np.concatenate(
        [np.asarray(r["out"]).transpose(2, 0, 1).reshape(TPC, C)
         for r in res.results], axis=0)
    return out.reshape(B, T, C)
